# revision 13
# baseline (speedup 1.0000x reference)
"""VQ-VAE decoder (vq_codebook) on 8 TRN2 NeuronCores, batch-sharded.

Pipeline per core (2 batch elements = 2048 tokens):
  1. VQ: scores = 2*ze.c - ||c||^2 via bf16 matmul (n2 folded in as two extra
     bf16 contraction rows), fp16 scores -> DVE max/max_index top-8, exact
     fp32 rescore of top-4 candidates (gather + affine_mul_reduce), gather
     winning codebook row, add pos_emb.
  2. 4 transformer blocks (pre-LN MHA + pre-LN FFN), bf16 matmuls:
     - LN stats on DVE (bn_stats/bn_aggr), rstd via exp(-0.5*ln(var+eps)),
       LN gamma/beta folded into the following weights host-side.
     - h -> hT via DMA xbar transposes; qkvT layout (features on partitions).
     - Attention: scoresT orientation, 4-head row-tiled QK matmuls, ACT exp
       (softmax max-subtraction skipped; scores are O(1)), col-tiled PV and
       ones-matmul row sums, output normalized by reciprocal sums.
  3. Output projection x @ Wout + bout -> logits fp32.

Self-contained: hardcodes all shapes; host preps transposed/augmented/bf16
operands and LN-folded weights; runs SPMD on cores 0-7 and concatenates.
"""

import numpy as np
import ml_dtypes

import concourse.bass as bass
import concourse.bacc as bacc
import concourse.mybir as mybir
from concourse import tile
from concourse import bass_utils

F32 = mybir.dt.float32
F16 = mybir.dt.float16
BF16 = mybir.dt.bfloat16
U32 = mybir.dt.uint32
AF = mybir.ActivationFunctionType
ALU = mybir.AluOpType

D_MODEL, D_PATCH, K_CODES, SEQLEN = 256, 768, 8192, 1024
N_HEADS, N_BLOCKS, DFF = 8, 4, 1024
BATCH, N_CORES = 16, 8
LN_EPS = 1e-5
B_PER_CORE = BATCH // N_CORES            # 2
TOKS = B_PER_CORE * SEQLEN               # 2048
NT = TOKS // 128                         # 16 token tiles
TPB = SEQLEN // 128                      # 8 token tiles per batch element
NCAND = 4                                # exact-rescore candidates
CAUG = 264                               # padded augmented row width (256+1 pad to 8)

_BUILD_CACHE = {}


def _bf16(x):
    return np.asarray(x, np.float32).astype(ml_dtypes.bfloat16)


def build_nc():
    nc = bacc.Bacc("TRN2", target_bir_lowering=False)

    # ---------------- DRAM I/O ----------------
    zet16 = nc.dram_tensor("zet16", [258, TOKS], BF16, kind="ExternalInput")
    zeaug = nc.dram_tensor("zeaug", [TOKS, CAUG], F32, kind="ExternalInput")
    cbt16 = nc.dram_tensor("cbt16", [258, K_CODES], BF16, kind="ExternalInput")
    cbaug = nc.dram_tensor("cbaug", [K_CODES, CAUG], F32, kind="ExternalInput")
    pos2 = nc.dram_tensor("pos2", [TOKS, D_MODEL], F32, kind="ExternalInput")
    wqk = nc.dram_tensor("wqk", [N_BLOCKS, D_MODEL, 512], BF16, kind="ExternalInput")
    bqk = nc.dram_tensor("bqk", [N_BLOCKS, 512, 1], F32, kind="ExternalInput")
    wv = nc.dram_tensor("wv", [N_BLOCKS, D_MODEL, D_MODEL], BF16, kind="ExternalInput")
    bv16 = nc.dram_tensor("bv16", [N_BLOCKS, 1, D_MODEL], BF16, kind="ExternalInput")
    wo = nc.dram_tensor("wo", [N_BLOCKS, D_MODEL, D_MODEL], BF16, kind="ExternalInput")
    bo16 = nc.dram_tensor("bo16", [N_BLOCKS, 1, D_MODEL], BF16, kind="ExternalInput")
    w1 = nc.dram_tensor("w1", [N_BLOCKS, D_MODEL, DFF], BF16, kind="ExternalInput")
    b1 = nc.dram_tensor("b1", [N_BLOCKS, DFF, 1], F32, kind="ExternalInput")
    w2 = nc.dram_tensor("w2", [N_BLOCKS, DFF, D_MODEL], BF16, kind="ExternalInput")
    b216 = nc.dram_tensor("b216", [N_BLOCKS, 1, D_MODEL], BF16, kind="ExternalInput")
    wout = nc.dram_tensor("wout", [D_MODEL, D_PATCH], BF16, kind="ExternalInput")
    bout16 = nc.dram_tensor("bout16", [1, D_PATCH], BF16, kind="ExternalInput")
    logits = nc.dram_tensor("logits", [TOKS, D_PATCH], F32, kind="ExternalOutput")

    with tile.TileContext(nc) as tc:
        with (
            tc.tile_pool(name="resident", bufs=1) as res,
            tc.tile_pool(name="smalls", bufs=4) as sm,
        ):
            # residual stream x: [128, 16 tiles x 256] fp32
            xall = res.tile([128, NT * D_MODEL], F32)
            ones16 = res.tile([1, 128], BF16)
            nc.vector.memset(ones16[:], 1.0)
            ones32 = res.tile([128, 32], BF16)
            nc.vector.memset(ones32[:], 1.0)

            # ================= Phase 1: VQ =================
            with (
                tc.tile_pool(name="vq_sb", bufs=1) as vqs,
                tc.tile_pool(name="vq_sc", bufs=2) as vsc,
                tc.tile_pool(name="vq_sm", bufs=4) as vsm,
                tc.tile_pool(name="vq_ps", bufs=2, space="PSUM") as vqp,
            ):
                cb0 = vqs.tile([128, K_CODES], BF16)
                cb1 = vqs.tile([128, K_CODES], BF16)
                cb2 = vqs.tile([2, K_CODES], BF16)
                nc.sync.dma_start(cb0[:], cbt16[0:128, :])
                nc.sync.dma_start(cb1[:], cbt16[128:256, :])
                nc.sync.dma_start(cb2[:], cbt16[256:258, :])
                zt0 = vqs.tile([128, TOKS], BF16)
                zt1 = vqs.tile([128, TOKS], BF16)
                zt2 = vqs.tile([2, TOKS], BF16)
                nc.sync.dma_start(zt0[:], zet16[0:128, :])
                nc.sync.dma_start(zt1[:], zet16[128:256, :])
                nc.sync.dma_start(zt2[:], zet16[256:258, :])
                zea = vqs.tile([128, NT * CAUG], F32)
                nc.sync.dma_start(
                    zea[:].rearrange("p (t d) -> p t d", d=CAUG),
                    zeaug[:].rearrange("(t p) d -> p t d", p=128))
                posb = vqs.tile([128, NT * D_MODEL], F32)
                nc.sync.dma_start(
                    posb[:].rearrange("p (t d) -> p t d", d=D_MODEL),
                    pos2[:].rearrange("(t p) d -> p t d", p=128))

                for t in range(NT):
                    tsl = slice(t * 128, t * 128 + 128)
                    sc16 = vsc.tile([128, K_CODES], F16, tag="sc16")
                    for qtr in range(4):
                        ps = vqp.tile([128, 2048], F32, tag="vq", name=f"vps{t}_{qtr}")
                        for ch in range(4):
                            c0 = qtr * 2048 + ch * 512
                            o = ps[:, ch * 512:ch * 512 + 512]
                            nc.tensor.matmul(o, zt0[:, tsl], cb0[:, c0:c0 + 512],
                                             start=True, stop=False)
                            nc.tensor.matmul(o, zt1[:, tsl], cb1[:, c0:c0 + 512],
                                             start=False, stop=False)
                            nc.tensor.matmul(o, zt2[:, tsl], cb2[:, c0:c0 + 512],
                                             start=False, stop=True)
                        nc.scalar.copy(sc16[:, qtr * 2048:qtr * 2048 + 2048], ps[:])
                    m8 = vsm.tile([128, 8], F16, tag="m8")
                    i8 = vsm.tile([128, 8], U32, tag="i8")
                    nc.vector.max(out=m8[:], in_=sc16[:])
                    nc.vector.max_index(out=i8[:], in_max=m8[:], in_values=sc16[:])
                    # exact rescore of top-NCAND candidates
                    cs = vsm.tile([128, NCAND], F32, tag="cs")
                    cif = vsm.tile([128, NCAND], F32, tag="cif")
                    for k in range(NCAND):
                        idx1 = vsm.tile([128, 1], U32, tag=f"idx1_{k}", name=f"idx1_{t}_{k}")
                        nc.vector.tensor_copy(idx1[:], i8[:, k:k + 1])
                        nc.vector.tensor_copy(cif[:, k:k + 1], idx1[:])  # u32 -> f32
                        gat = vsm.tile([128, CAUG], F32, tag=f"gat{k}", name=f"gat_{t}_{k}")
                        nc.gpsimd.indirect_dma_start(
                            out=gat[:], out_offset=None, in_=cbaug[:],
                            in_offset=bass.IndirectOffsetOnAxis(ap=idx1[:, :1], axis=0))
                        scr = vsm.tile([128, CAUG], BF16, tag=f"scr{k}", name=f"scr_{t}_{k}")
                        nc.vector.affine_mul_reduce(
                            out=scr[:], accum_out=cs[:, k:k + 1],
                            in0=zea[:, t * CAUG:(t + 1) * CAUG], in1=gat[:],
                            scale=1.0, bias=0.0)
                    # pick best (ties -> smallest code index, matching argmin)
                    best = vsm.tile([128, 1], F32, tag="best")
                    nc.vector.reduce_max(best[:], cs[:], axis=mybir.AxisListType.X)
                    eq = vsm.tile([128, NCAND], F32, tag="eq")
                    nc.vector.tensor_scalar(eq[:], cs[:], best[:, :1], None, op0=ALU.is_ge)
                    msk = vsm.tile([128, NCAND], F32, tag="msk")
                    nc.vector.tensor_scalar(msk[:], eq[:], -1e9, 1e9,
                                            op0=ALU.mult, op1=ALU.add)
                    nc.vector.tensor_tensor(out=msk[:], in0=msk[:], in1=cif[:], op=ALU.add)
                    bidxf = vsm.tile([128, 1], F32, tag="bidxf")
                    nc.vector.tensor_reduce(bidxf[:], msk[:], axis=mybir.AxisListType.X,
                                            op=ALU.min)
                    bidx = vsm.tile([128, 1], U32, tag="bidx")
                    nc.vector.tensor_copy(bidx[:], bidxf[:])  # f32 -> u32
                    zq = vsm.tile([128, CAUG], F32, tag="zq")
                    nc.gpsimd.indirect_dma_start(
                        out=zq[:], out_offset=None, in_=cbaug[:],
                        in_offset=bass.IndirectOffsetOnAxis(ap=bidx[:, :1], axis=0))
                    # x = zq + pos
                    nc.vector.tensor_tensor(
                        out=xall[:, t * 256:(t + 1) * 256], in0=zq[:, :256],
                        in1=posb[:, t * 256:(t + 1) * 256], op=ALU.add)

            # ================= Phase 2: transformer blocks =================
            with (
                tc.tile_pool(name="blk_w", bufs=2) as bw,
                tc.tile_pool(name="blk_act", bufs=1) as ba,
                tc.tile_pool(name="blk_probs", bufs=3) as bpr,
                tc.tile_pool(name="blk_sm", bufs=4) as bsm,
            ):
                for blk in range(N_BLOCKS):
                    # ---- load weights for this block
                    wqk_sb = [bw.tile([128, 512], BF16, tag=f"wqk{d}", name=f"wqk_{blk}_{d}") for d in range(2)]
                    wv_sb = [bw.tile([128, 256], BF16, tag=f"wv{d}", name=f"wv_{blk}_{d}") for d in range(2)]
                    wo_sb = [bw.tile([128, 256], BF16, tag=f"wo{d}", name=f"wo_{blk}_{d}") for d in range(2)]
                    w1_sb = [bw.tile([128, DFF], BF16, tag=f"w1{d}", name=f"w1_{blk}_{d}") for d in range(2)]
                    w2_sb = [bw.tile([128, 256], BF16, tag=f"w2{d}", name=f"w2_{blk}_{d}") for d in range(8)]
                    for d in range(2):
                        nc.sync.dma_start(wqk_sb[d][:], wqk[blk, d * 128:(d + 1) * 128, :])
                        nc.sync.dma_start(wv_sb[d][:], wv[blk, d * 128:(d + 1) * 128, :])
                        nc.sync.dma_start(wo_sb[d][:], wo[blk, d * 128:(d + 1) * 128, :])
                        nc.sync.dma_start(w1_sb[d][:], w1[blk, d * 128:(d + 1) * 128, :])
                    for d in range(8):
                        nc.sync.dma_start(w2_sb[d][:], w2[blk, d * 128:(d + 1) * 128, :])
                    bqk_sb = [bw.tile([128, 1], F32, tag=f"bqk{m}", name=f"bqk_{blk}_{m}") for m in range(4)]
                    for m in range(4):
                        nc.sync.dma_start(bqk_sb[m][:], bqk[blk, m * 128:(m + 1) * 128, :])
                    b1_sb = [bw.tile([128, 1], F32, tag=f"b1{p}", name=f"b1_{blk}_{p}") for p in range(8)]
                    for p in range(8):
                        nc.sync.dma_start(b1_sb[p][:], b1[blk, p * 128:(p + 1) * 128, :])
                    bv_sb = bw.tile([1, 256], BF16, tag="bv", name=f"bv_{blk}")
                    bo_sb = bw.tile([1, 256], BF16, tag="bo", name=f"bo_{blk}")
                    b2_sb = bw.tile([1, 256], BF16, tag="b2", name=f"b2_{blk}")
                    nc.sync.dma_start(bv_sb[:], bv16[blk])
                    nc.sync.dma_start(bo_sb[:], bo16[blk])
                    nc.sync.dma_start(b2_sb[:], b216[blk])

                    # ---- LN1 -> h16 (bf16) ; gamma/beta folded into weights
                    h16 = ba.tile([128, NT * 256], BF16, tag="h16", name=f"h16_{blk}")
                    _layernorm(nc, tc, bsm, xall, h16, blk, "ln1")
                    # ---- hT via xbar transposes
                    hT = [ba.tile([128, TOKS], BF16, tag=f"hT{d}", name=f"hT_{blk}_{d}") for d in range(2)]
                    for t in range(NT):
                        for d in range(2):
                            nc.sync.dma_start_transpose(
                                hT[d][:, t * 128:(t + 1) * 128],
                                h16[:, t * 256 + d * 128: t * 256 + (d + 1) * 128])

                    # ---- qkT (feature-major) and v (token-major)
                    qkT = [ba.tile([128, TOKS], BF16, tag=f"qkT{m}", name=f"qkT_{blk}_{m}") for m in range(4)]
                    v16 = ba.tile([128, NT * 256], BF16, tag="v16", name=f"v16_{blk}")
                    with tc.tile_pool(name=f"qkv_ps_{blk}", bufs=4, space="PSUM") as qp:
                        for m in range(4):
                            for chv in range(4):
                                c0 = chv * 512
                                pq = qp.tile([128, 512], F32, tag="pq", name=f"pq_{blk}_{m}_{chv}")
                                nc.tensor.matmul(pq[:], wqk_sb[0][:, m * 128:(m + 1) * 128],
                                                 hT[0][:, c0:c0 + 512], start=True, stop=False)
                                nc.tensor.matmul(pq[:], wqk_sb[1][:, m * 128:(m + 1) * 128],
                                                 hT[1][:, c0:c0 + 512], start=False, stop=True)
                                nc.vector.tensor_scalar(qkT[m][:, c0:c0 + 512], pq[:],
                                                        bqk_sb[m][:, :1], None, op0=ALU.add)
                        for t in range(NT):
                            tsl = slice(t * 128, t * 128 + 128)
                            pv = qp.tile([128, 256], F32, tag="pv", name=f"pv_{blk}_{t}")
                            nc.tensor.matmul(pv[:], hT[0][:, tsl], wv_sb[0][:], start=True, stop=False)
                            nc.tensor.matmul(pv[:], hT[1][:, tsl], wv_sb[1][:], start=False, stop=False)
                            nc.tensor.matmul(pv[:], ones16[:, :128], bv_sb[:], start=False, stop=True)
                            nc.vector.tensor_copy(v16[:, t * 256:(t + 1) * 256], pv[:])

                    # ---- attention: 2 batch x 2 groups of 4 heads
                    oT16 = [ba.tile([128, SEQLEN], BF16, tag=f"oT{i}", name=f"oT_{blk}_{i}")
                            for i in range(4)]  # (b, g) -> [4heads*32, 1024 q]
                    for b in range(2):
                        for g in range(2):
                            _attn_group(nc, tc, bpr, bsm, qkT, v16, ones32,
                                        oT16[b * 2 + g], blk, b, g)

                    # ---- S3: x += oT @ Wo + bo
                    with tc.tile_pool(name=f"s3_ps_{blk}", bufs=4, space="PSUM") as s3p:
                        for t in range(NT):
                            b, q = t // TPB, (t % TPB) * 128
                            po = s3p.tile([128, 256], F32, tag="po", name=f"po_{blk}_{t}")
                            for g in range(2):
                                nc.tensor.matmul(po[:], oT16[b * 2 + g][:, q:q + 128],
                                                 wo_sb[g][:], start=(g == 0), stop=False)
                            nc.tensor.matmul(po[:], ones16[:, :128], bo_sb[:],
                                             start=False, stop=True)
                            nc.vector.tensor_tensor(
                                out=xall[:, t * 256:(t + 1) * 256],
                                in0=xall[:, t * 256:(t + 1) * 256], in1=po[:], op=ALU.add)

                    # ---- LN2 -> h2 -> h2T
                    h2 = ba.tile([128, NT * 256], BF16, tag="h16", name=f"h2_{blk}")
                    _layernorm(nc, tc, bsm, xall, h2, blk, "ln2")
                    h2T = [ba.tile([128, TOKS], BF16, tag=f"hT{d}", name=f"h2T_{blk}_{d}") for d in range(2)]
                    for t in range(NT):
                        for d in range(2):
                            nc.sync.dma_start_transpose(
                                h2T[d][:, t * 128:(t + 1) * 128],
                                h2[:, t * 256 + d * 128: t * 256 + (d + 1) * 128])

                    # ---- FFN: gT = gelu(W1T h2T + b1) ; x += gT.T @ W2 + b2
                    gT = [ba.tile([128, TOKS], BF16, tag=f"gT{p}", name=f"gT_{blk}_{p}") for p in range(8)]
                    with tc.tile_pool(name=f"ffn1_ps_{blk}", bufs=2, space="PSUM") as f1p:
                        for p in range(8):
                            for hh in range(2):
                                pu = f1p.tile([128, 1024], F32, tag="pu", name=f"pu_{blk}_{p}_{hh}")
                                for ch in range(2):
                                    c0 = hh * 1024 + ch * 512
                                    o = pu[:, ch * 512:ch * 512 + 512]
                                    nc.tensor.matmul(o, w1_sb[0][:, p * 128:(p + 1) * 128],
                                                     h2T[0][:, c0:c0 + 512], start=True, stop=False)
                                    nc.tensor.matmul(o, w1_sb[1][:, p * 128:(p + 1) * 128],
                                                     h2T[1][:, c0:c0 + 512], start=False, stop=True)
                                nc.scalar.activation(gT[p][:, hh * 1024:(hh + 1) * 1024], pu[:],
                                                     AF.Gelu_apprx_tanh, bias=b1_sb[p][:, :1])
                    with tc.tile_pool(name=f"ffn2_ps_{blk}", bufs=4, space="PSUM") as f2p:
                        for t in range(NT):
                            tsl = slice(t * 128, t * 128 + 128)
                            pf = f2p.tile([128, 256], F32, tag="pf", name=f"pf_{blk}_{t}")
                            for p in range(8):
                                nc.tensor.matmul(pf[:], gT[p][:, tsl], w2_sb[p][:],
                                                 start=(p == 0), stop=False)
                            nc.tensor.matmul(pf[:], ones16[:, :128], b2_sb[:],
                                             start=False, stop=True)
                            nc.vector.tensor_tensor(
                                out=xall[:, t * 256:(t + 1) * 256],
                                in0=xall[:, t * 256:(t + 1) * 256], in1=pf[:], op=ALU.add)

                # ================= Phase 3: output projection =================
                wout_sb = [bw.tile([128, D_PATCH], BF16, tag=f"wout{d}", name=f"wout_{d}") for d in range(2)]
                for d in range(2):
                    nc.sync.dma_start(wout_sb[d][:], wout[d * 128:(d + 1) * 128, :])
                bout_sb = bw.tile([1, D_PATCH], BF16, tag="bout")
                nc.sync.dma_start(bout_sb[:], bout16[:])
                x16 = ba.tile([128, NT * 256], BF16, tag="h16", name="x16_out")
                for t in range(NT):
                    nc.vector.tensor_copy(x16[:, t * 256:(t + 1) * 256],
                                          xall[:, t * 256:(t + 1) * 256])
                xT = [ba.tile([128, TOKS], BF16, tag=f"hT{d}", name=f"xT_{d}") for d in range(2)]
                for t in range(NT):
                    for d in range(2):
                        nc.sync.dma_start_transpose(
                            xT[d][:, t * 128:(t + 1) * 128],
                            x16[:, t * 256 + d * 128: t * 256 + (d + 1) * 128])
                with tc.tile_pool(name="out_ps", bufs=2, space="PSUM") as op:
                    for t in range(NT):
                        tsl = slice(t * 128, t * 128 + 128)
                        pl = op.tile([128, 1024], F32, tag="pl", name=f"pl_{t}")
                        for ch, w in ((0, 512), (1, 256)):
                            o = pl[:, ch * 512:ch * 512 + w]
                            nc.tensor.matmul(o, xT[0][:, tsl], wout_sb[0][:, ch * 512:ch * 512 + w],
                                             start=True, stop=False)
                            nc.tensor.matmul(o, xT[1][:, tsl], wout_sb[1][:, ch * 512:ch * 512 + w],
                                             start=False, stop=False)
                            nc.tensor.matmul(o, ones16[:, :128], bout_sb[:, ch * 512:ch * 512 + w],
                                             start=False, stop=True)
                        ol = ba.tile([128, D_PATCH], F32, tag="ol", name=f"ol_{t}")
                        nc.scalar.copy(ol[:], pl[:, :768])
                        nc.sync.dma_start(logits[t * 128:(t + 1) * 128, :], ol[:])

    nc.compile()
    return nc


def _layernorm(nc, tc, bsm, xall, h16, blk, tag):
    """h16 = (x - mean(x)) * rsqrt(var(x) + eps), cast bf16.
    rstd computed as exp(-0.5 * ln(var + eps)) to stay in one ACT table set."""
    epsc = bsm.tile([128, 1], F32, tag="lneps", name=f"{tag}eps_{blk}")
    nc.vector.memset(epsc[:], LN_EPS)
    meancol = bsm.tile([128, NT], F32, tag="lnmean", name=f"{tag}mean_{blk}")
    varcol = bsm.tile([128, NT], F32, tag="lnvar", name=f"{tag}var_{blk}")
    rstdcol = bsm.tile([128, NT], F32, tag="lnrstd", name=f"{tag}rstd_{blk}")
    lncol = bsm.tile([128, NT], F32, tag="lnln", name=f"{tag}ln_{blk}")
    for t in range(NT):
        stats = bsm.tile([128, 6], F32, tag="lnstats", name=f"{tag}stats_{blk}_{t}")
        mv = bsm.tile([128, 2], F32, tag="lnmv", name=f"{tag}mv_{blk}_{t}")
        nc.vector.bn_stats(stats[:], xall[:, t * 256:(t + 1) * 256])
        nc.vector.bn_aggr(mv[:], stats[:])
        nc.vector.tensor_copy(meancol[:, t:t + 1], mv[:, 0:1])
        nc.vector.tensor_copy(varcol[:, t:t + 1], mv[:, 1:2])
    nc.scalar.activation(lncol[:], varcol[:], AF.Ln, bias=epsc[:, :1])
    nc.scalar.activation(rstdcol[:], lncol[:], AF.Exp, scale=-0.5)
    for t in range(NT):
        nc.vector.tensor_scalar(
            h16[:, t * 256:(t + 1) * 256], xall[:, t * 256:(t + 1) * 256],
            meancol[:, t:t + 1], rstdcol[:, t:t + 1],
            op0=ALU.subtract, op1=ALU.mult)


def _attn_group(nc, tc, bpr, bsm, qkT, v16, ones32, oT_out, blk, b, g):
    """One (batch, 4-head-group): scoresT QK matmuls -> exp -> PV + row sums
    -> normalize. oT_out: [128 (4h x 32 dims), 1024 q] bf16."""
    name = f"{blk}_{b}_{g}"
    with (
        tc.tile_pool(name=f"s1_ps_{name}", bufs=1, space="PSUM") as s1p,
        tc.tile_pool(name=f"acc_ps_{name}", bufs=1, space="PSUM") as accp,
    ):
        otp = accp.tile([128, SEQLEN], F32, tag="otp", name=f"otp_{name}")
        smp = accp.tile([128, SEQLEN], F32, tag="smp", name=f"smp_{name}")
        qt, kt = qkT[g], qkT[2 + g]
        tok0 = b * SEQLEN

        for kk in range(8):
            for qc in range(2):
                ps = s1p.tile([128, 2048], F32, tag="s1", name=f"s1_{name}_{kk}_{qc}")
                for h in range(4):
                    nc.tensor.matmul(
                        ps[:, h * 512:(h + 1) * 512],
                        kt[32 * h:32 * h + 32, tok0 + kk * 128: tok0 + (kk + 1) * 128],
                        qt[32 * h:32 * h + 32, tok0 + qc * 512: tok0 + (qc + 1) * 512],
                        start=True, stop=True, tile_position=(32 * h, 0))
                pr = bpr.tile([128, 2048], BF16, tag="probs", name=f"pr_{name}_{kk}_{qc}")
                nc.scalar.activation(pr[:], ps[:], AF.Exp)
                qs = slice(qc * 512, qc * 512 + 512)
                vbase = (b * TPB + kk) * 256 + g * 128
                for h in range(4):
                    phs = pr[:, h * 512:(h + 1) * 512]
                    nc.tensor.matmul(otp[32 * h:32 * h + 32, qs],
                                     v16[:, vbase + h * 32: vbase + (h + 1) * 32],
                                     phs, start=(kk == 0), stop=(kk == 7),
                                     tile_position=(0, 32 * h))
                    nc.tensor.matmul(smp[32 * h:32 * h + 32, qs],
                                     ones32[:, :], phs, start=(kk == 0), stop=(kk == 7),
                                     tile_position=(0, 32 * h))
        # normalize: oT = otp * (1 / sums)
        rec = bsm.tile([128, SEQLEN], BF16, tag="rec", name=f"rec_{name}")
        with nc.allow_low_precision(reason="softmax recip consumed by bf16 probs"):
            nc.vector.reciprocal(rec[:], smp[:])
        nc.vector.tensor_tensor(out=oT_out[:], in0=otp[:], in1=rec[:], op=ALU.mult)


def _prep_inputs(inputs):
    """Host-side prep: shard, transpose, fold LN scales, cast."""
    ze = np.asarray(inputs["ze"], np.float32)
    cb = np.asarray(inputs["codebook"], np.float32)
    pos = np.asarray(inputs["pos_emb"], np.float32)
    n2 = (cb * cb).sum(-1)
    n2h = n2.astype(ml_dtypes.bfloat16).astype(np.float32)
    n2l = (n2 - n2h).astype(ml_dtypes.bfloat16).astype(np.float32)

    cbt = np.empty((258, K_CODES), np.float32)
    cbt[:256] = cb.T
    cbt[256] = -n2h
    cbt[257] = -n2l
    cbt16 = _bf16(cbt)
    cbaug = np.zeros((K_CODES, CAUG), np.float32)
    cbaug[:, :256] = cb
    cbaug[:, 256] = n2
    pos2 = np.concatenate([pos] * B_PER_CORE, axis=0)

    shared = {"cbt16": cbt16, "cbaug": cbaug, "pos2": pos2}

    sq = 1.0 / np.sqrt(np.float32(D_MODEL // N_HEADS))
    wqk_l, bqk_l, wv_l, bv_l, wo_l, bo_l = [], [], [], [], [], []
    w1_l, b1_l, w2_l, b2_l = [], [], [], []
    for i in range(N_BLOCKS):
        Wqkv = np.asarray(inputs["Wqkv"][i], np.float32)
        bqkv = np.asarray(inputs["bqkv"][i], np.float32)
        s1v = np.asarray(inputs["ln1_s"][i], np.float32)
        b1v = np.asarray(inputs["ln1_b"][i], np.float32)
        Wf = s1v[:, None] * Wqkv
        bf = bqkv + b1v @ Wqkv
        Wf[:, :256] *= sq
        bf[:256] *= sq
        wqk_l.append(_bf16(Wf[:, :512]))
        bqk_l.append(bf[:512].astype(np.float32)[:, None])
        wv_l.append(_bf16(Wf[:, 512:]))
        bv_l.append(_bf16(bf[512:][None, :]))
        wo_l.append(_bf16(inputs["Wo"][i]))
        bo_l.append(_bf16(np.asarray(inputs["bo"][i], np.float32)[None, :]))
        W1 = np.asarray(inputs["W1"][i], np.float32)
        s2v = np.asarray(inputs["ln2_s"][i], np.float32)
        b2v = np.asarray(inputs["ln2_b"][i], np.float32)
        w1_l.append(_bf16(s2v[:, None] * W1))
        b1_l.append((np.asarray(inputs["b1"][i], np.float32) + b2v @ W1).astype(np.float32)[:, None])
        w2_l.append(_bf16(inputs["W2"][i]))
        b2_l.append(_bf16(np.asarray(inputs["b2"][i], np.float32)[None, :]))
    shared.update({
        "wqk": np.stack(wqk_l), "bqk": np.stack(bqk_l),
        "wv": np.stack(wv_l), "bv16": np.stack(bv_l),
        "wo": np.stack(wo_l), "bo16": np.stack(bo_l),
        "w1": np.stack(w1_l), "b1": np.stack(b1_l),
        "w2": np.stack(w2_l), "b216": np.stack(b2_l),
        "wout": _bf16(inputs["Wout"]),
        "bout16": _bf16(np.asarray(inputs["bout"], np.float32)[None, :]),
    })

    in_maps = []
    for c in range(N_CORES):
        zec = ze[c * B_PER_CORE:(c + 1) * B_PER_CORE].reshape(TOKS, D_MODEL)
        zet = np.empty((258, TOKS), np.float32)
        zet[:256] = (2.0 * zec).T
        zet[256:258] = 1.0
        zeaug = np.zeros((TOKS, CAUG), np.float32)
        zeaug[:, :256] = 2.0 * zec
        zeaug[:, 256] = -1.0
        in_maps.append({**shared, "zet16": _bf16(zet), "zeaug": zeaug})
    return in_maps


def kernel(**inputs) -> np.ndarray:
    if "nc" not in _BUILD_CACHE:
        _BUILD_CACHE["nc"] = build_nc()
    nc = _BUILD_CACHE["nc"]
    in_maps = _prep_inputs(inputs)
    res = bass_utils.run_bass_kernel_spmd(nc, in_maps, core_ids=list(range(N_CORES)))
    out = np.stack([res.results[c]["logits"] for c in range(N_CORES)])
    return out.reshape(BATCH, SEQLEN, D_PATCH)
